# revision 4
# baseline (speedup 1.0000x reference)
"""Bass/Trainium2 kernel for the attention module (nn_Attention_18648747999422).

Computes, for B=32, S=2048, ENC=DEC=1024:
    h_proj  = hidden @ w_h.T                     [b, dec]
    e_proj  = encoder_outputs @ w_e.T            [b, s, dec]
    energy  = tanh(h_proj + e_proj + attn_b)
    logits  = energy @ v_w                       [b, s]
    attw    = softmax(where(mask==0, -1e10, logits), axis=s)
    context = attw @ encoder_outputs             [b, enc]
Returns (context, attw).

Data-parallel over 8 NeuronCores: batch dim sharded 4 per core, params
replicated. The big matmul (e_proj) contracts over enc, so encoder_outputs is
cast fp32->fp16 during the DMA load and transposed on-chip with the DMA xbar
(2-byte dtype requirement is why the matmul path runs in fp16; PSUM
accumulation stays fp32, softmax stays fp32).
"""

from contextlib import ExitStack

import numpy as np

B, S, ENC, DEC = 32, 2048, 1024, 1024
NCORES = 8
B_LOC = B // NCORES  # batches per core
P = 128
ET = ENC // P   # 8 enc k-tiles
DT = DEC // P   # 8 dec m-tiles
ST = S // P     # 16 s-tiles
SC = S // 512   # 4 s-chunks of 512
NEG = -1e10

MM_DT_NAME = "float16"   # matmul operand dtype

_CACHE = {}


def _build():
    from concourse import bacc
    import concourse.tile as tile
    import concourse.mybir as mybir

    f32 = mybir.dt.float32
    f16 = getattr(mybir.dt, MM_DT_NAME)
    TANH = mybir.ActivationFunctionType.Tanh
    EXP = mybir.ActivationFunctionType.Exp

    nc = bacc.Bacc("TRN2", target_bir_lowering=False, debug=False)

    enc_in = nc.dram_tensor("enc_in", [B_LOC, S, ENC], f32, kind="ExternalInput")
    hidT_in = nc.dram_tensor("hidT_in", [P, DT, B_LOC], f16, kind="ExternalInput")
    maskb_in = nc.dram_tensor("maskb_in", [B_LOC, S], f32, kind="ExternalInput")
    w_eT_in = nc.dram_tensor("w_eT_in", [P, ET, DEC], f16, kind="ExternalInput")
    w_hT_in = nc.dram_tensor("w_hT_in", [P, DT, DEC], f16, kind="ExternalInput")
    v_wT_in = nc.dram_tensor("v_wT_in", [P, DT], f16, kind="ExternalInput")
    a_bT_in = nc.dram_tensor("a_bT_in", [P, DT], f32, kind="ExternalInput")
    ctx_out = nc.dram_tensor("ctx_out", [B_LOC, ENC], f32, kind="ExternalOutput")
    attw_out = nc.dram_tensor("attw_out", [B_LOC, S], f32, kind="ExternalOutput")

    with tile.TileContext(nc) as tc, ExitStack() as es:
        singles = es.enter_context(tc.tile_pool(name="singles", bufs=1))

        # ---- constants that live for the whole kernel ----
        w_e_sb = singles.tile([P, ET, DEC], f16)
        v_w_sb = singles.tile([P, DT], f16)
        a_bT_sb = singles.tile([P, DT], f32)
        hb_sb = singles.tile([P, DT, B_LOC], f32)

        nc.sync.dma_start(w_e_sb[:], w_eT_in[:])
        nc.sync.dma_start(v_w_sb[:], v_wT_in[:])
        nc.sync.dma_start(a_bT_sb[:], a_bT_in[:])

        # ---- h_proj (scoped pools, freed before the main phase) ----
        # hb[k, dt, b] = sum_d w_h[k,d] hidden[b,d] + attn_b[k]
        with (
            tc.tile_pool(name="htmp", bufs=1) as htmp,
            tc.tile_pool(name="psum_h", bufs=1, space="PSUM") as h_pool,
        ):
            w_h_sb = htmp.tile([P, DT, DEC], f16)
            hidT_sb = htmp.tile([P, DT, B_LOC], f16)
            nc.sync.dma_start(w_h_sb[:], w_hT_in[:])
            nc.sync.dma_start(hidT_sb[:], hidT_in[:])
            psum_h = h_pool.tile([P, DT, B_LOC], f32)
            for kt in range(DT):
                for dh in range(DT):
                    nc.tensor.matmul(
                        psum_h[:, kt, :],
                        w_h_sb[:, dh, kt * P:(kt + 1) * P],
                        hidT_sb[:, dh, :],
                        start=(dh == 0),
                        stop=(dh == DT - 1),
                    )
            nc.vector.tensor_tensor(
                hb_sb[:],
                psum_h[:],
                a_bT_sb[:, :, None].to_broadcast((P, DT, B_LOC)),
                mybir.AluOpType.add,
            )

        # ---- main-phase pools ----
        nat_pool = es.enter_context(tc.tile_pool(name="nat", bufs=2))
        encT_pool = es.enter_context(tc.tile_pool(name="encT", bufs=3))
        energy_pool = es.enter_context(tc.tile_pool(name="energy", bufs=3))
        rows2 = es.enter_context(tc.tile_pool(name="rows2", bufs=2))
        rows1 = es.enter_context(tc.tile_pool(name="rows1", bufs=1))
        pe_pool = es.enter_context(tc.tile_pool(name="psum_e", bufs=3, space="PSUM"))
        logit_pool = es.enter_context(
            tc.tile_pool(name="psum_logit", bufs=2, space="PSUM"))
        ctx_pool = es.enter_context(
            tc.tile_pool(name="psum_ctx", bufs=2, space="PSUM"))

        # pending logit matmul (lags one (sc,dt) step so the tanh that feeds
        # it overlaps the next group's e_proj matmuls instead of stalling PE)
        pending = []
        # per-batch deferred softmax+context state
        deferred = []

        def emit_logit(st_):
            sc_, dt_, en_, bstate = st_
            nc.tensor.matmul(
                bstate["psum_logit"][sc_],
                v_w_sb[:, dt_:dt_ + 1],
                en_[:],
                start=(dt_ == 0),
                stop=(dt_ == DT - 1),
            )
            if dt_ == DT - 1:
                # this s-chunk's logits are complete -> copy out of PSUM
                nc.vector.tensor_copy(
                    out=bstate["logits_row"][:, sc_ * 512:(sc_ + 1) * 512],
                    in_=bstate["psum_logit"][sc_],
                )
                bstate["psum_logit"][sc_] = None

        def emit_softmax_ctx(bstate):
            b = bstate["b"]
            logits_row = bstate["logits_row"]
            nat_b16 = bstate["nat_b16"]
            # mask: logits += maskbias (0 or -1e10)
            nc.vector.tensor_tensor(
                logits_row[:], logits_row[:], bstate["maskb_row"][:],
                mybir.AluOpType.add,
            )
            neg_max = rows1.tile([1, 1], f32, name="neg_max", tag="neg_max")
            nc.vector.tensor_reduce(
                neg_max[:], logits_row[:],
                axis=mybir.AxisListType.X, op=mybir.AluOpType.max, negate=True,
            )
            attw_row = rows1.tile([1, S], f32, name="attw_row", tag="attw_row")
            sumexp = rows1.tile([1, 1], f32, name="sumexp", tag="sumexp")
            nc.scalar.activation(
                out=attw_row[:], in_=logits_row[:], func=EXP,
                bias=neg_max[:], scale=1.0, accum_out=sumexp[:],
            )
            rsum = rows1.tile([1, 1], f32, name="rsum", tag="rsum")
            nc.vector.reciprocal(rsum[:], sumexp[:])
            # fp16 copy for the context matmul (scaled), fp32 copy for output
            attw16_row = rows1.tile([1, S], f16, name="attw16_row", tag="attw16_row")
            nc.vector.tensor_scalar_mul(attw16_row[:], attw_row[:], rsum[:])
            nc.vector.tensor_scalar_mul(attw_row[:], attw_row[:], rsum[:])
            nc.scalar.dma_start(attw_out[b:b + 1, :], attw_row[:])
            # attw fp16 [1,2048] -> [16,128] -> xbar transpose -> [128,16]
            attw16 = rows1.tile([ST, P], f16, name="attw16", tag="attw16")
            nc.scalar.dma_start(
                attw16[:],
                attw16_row[0:1, :].rearrange("p (c j) -> p c j", c=ST),
            )
            attwT = rows1.tile([P, ST], f16, name="attwT", tag="attwT")
            nc.scalar.dma_start(attwT[:], attw16[:], transpose=True)
            # context: ctx[n] = sum_s attw[s] * enc[b, s, n]
            ctx_row = rows1.tile([1, ENC], f32, name="ctx_row", tag="ctx_row")
            for nh in range(2):
                psum_ctx = ctx_pool.tile([1, 512], f32, name="psum_ctx",
                                         tag="psum_ctx")
                for st in range(ST):
                    nc.tensor.matmul(
                        psum_ctx[:],
                        attwT[:, st:st + 1],
                        nat_b16[:, st, nh * 512:(nh + 1) * 512],
                        start=(st == 0),
                        stop=(st == ST - 1),
                    )
                nc.vector.tensor_copy(
                    out=ctx_row[:, nh * 512:(nh + 1) * 512], in_=psum_ctx[:]
                )
            nc.scalar.dma_start(ctx_out[b:b + 1, :], ctx_row[:])

        for b in range(B_LOC):
            # ---- loads: cast fp32->fp16 during SWDGE DMA ----
            nat_b16 = nat_pool.tile([P, ST, ENC], f16, name="nat_b16", tag="nat")
            for g in range(4):
                nc.gpsimd.dma_start(
                    out=nat_b16[:, 4 * g:4 * (g + 1), :],
                    in_=enc_in[b, 512 * g:512 * (g + 1), :].rearrange(
                        "(st p) e -> p st e", p=P
                    ),
                )
            maskb_row = rows2.tile([1, S], f32, name="maskb_row", tag="maskb_row")
            nc.sync.dma_start(maskb_row[:], maskb_in[b:b + 1, :])

            bstate = {
                "b": b,
                "nat_b16": nat_b16,
                "maskb_row": maskb_row,
                "logits_row": rows2.tile([1, S], f32, name="logits_row",
                                         tag="logits_row"),
                "psum_logit": {},
            }

            # ---- per-s-chunk: xbar transpose then matmuls ----
            for sc in range(SC):
                encT_c = encT_pool.tile([P, ET, 512], f16, name="encT_c",
                                        tag="encT_c")
                for st in range(4):
                    for et in range(ET):
                        nc.sync.dma_start(
                            out=encT_c[:, et, st * P:(st + 1) * P],
                            in_=nat_b16[:, sc * 4 + st, et * P:(et + 1) * P],
                            transpose=True,
                        )
                for dt in range(DT):
                    psum_e = pe_pool.tile([P, 512], f32, name="psum_e",
                                          tag="psum_e")
                    for et in range(ET):
                        nc.tensor.matmul(
                            psum_e[:],
                            w_e_sb[:, et, dt * P:(dt + 1) * P],
                            encT_c[:, et, :],
                            start=(et == 0),
                            stop=(et == ET - 1),
                        )
                    energy = energy_pool.tile([P, 512], f16, name="energy",
                                              tag="energy")
                    nc.scalar.activation(
                        out=energy[:], in_=psum_e[:], func=TANH,
                        bias=hb_sb[:, dt, b:b + 1], scale=1.0,
                    )
                    if dt == 0:
                        bstate["psum_logit"][sc] = logit_pool.tile(
                            [1, 512], f32, name="psum_logit", tag="psum_logit"
                        )
                    if pending:
                        emit_logit(pending.pop())
                    pending.append((sc, dt, energy, bstate))
                if sc == 0 and deferred:
                    # hide the softmax+ctx latency of the previous batch
                    # behind this batch's matmuls
                    emit_softmax_ctx(deferred.pop())
            deferred.append(bstate)

        while pending:
            emit_logit(pending.pop())
        while deferred:
            emit_softmax_ctx(deferred.pop())

    nc.compile()
    return nc


def get_nc():
    if "nc" not in _CACHE:
        _CACHE["nc"] = _build()
    return _CACHE["nc"]


def host_prep(hidden, encoder_outputs, mask, attn_w, attn_b, v_w):
    """Shard + lay out inputs for the 8 cores. Returns list of in_maps."""
    np16 = np.float16 if MM_DT_NAME == "float16" else None
    if np16 is None:
        import ml_dtypes
        np16 = ml_dtypes.bfloat16

    def tile_kp(a):  # [K, M] -> [128, K//128, M]
        k, m = a.shape
        return np.ascontiguousarray(
            a.reshape(k // P, P, m).transpose(1, 0, 2)
        )

    w_h = attn_w[:, :DEC]          # [dec(k), dec(d)]
    w_e = attn_w[:, DEC:]          # [dec(k), enc(e)]
    w_hT = tile_kp(np.ascontiguousarray(w_h.T)).astype(np16)   # [128, 8, 1024]
    w_eT = tile_kp(np.ascontiguousarray(w_e.T)).astype(np16)   # [128, 8, 1024]
    v_wT = np.ascontiguousarray(v_w.reshape(DT, P).T).astype(np16)      # [128, 8]
    a_bT = np.ascontiguousarray(attn_b.reshape(DT, P).T).astype(np.float32)

    in_maps = []
    for c in range(NCORES):
        sl = slice(c * B_LOC, (c + 1) * B_LOC)
        hid = hidden[sl]                                       # [4, 1024]
        hidT = tile_kp(np.ascontiguousarray(hid.T)).astype(np16)  # [128, 8, 4]
        maskb = np.where(mask[sl] == 0, np.float32(NEG), np.float32(0.0)).astype(
            np.float32
        )
        in_maps.append({
            "enc_in": np.ascontiguousarray(encoder_outputs[sl]).astype(
                np.float32, copy=False
            ),
            "hidT_in": hidT,
            "maskb_in": maskb,
            "w_eT_in": w_eT,
            "w_hT_in": w_hT,
            "v_wT_in": v_wT,
            "a_bT_in": a_bT,
        })
    return in_maps


def kernel(hidden, encoder_outputs, mask, attn_w, attn_b, v_w):
    from concourse.bass_utils import run_bass_kernel_spmd

    hidden = np.asarray(hidden)
    encoder_outputs = np.asarray(encoder_outputs)
    mask = np.asarray(mask)
    attn_w = np.asarray(attn_w)
    attn_b = np.asarray(attn_b)
    v_w = np.asarray(v_w)

    nc = get_nc()
    in_maps = host_prep(hidden, encoder_outputs, mask, attn_w, attn_b, v_w)
    res = run_bass_kernel_spmd(nc, in_maps, core_ids=list(range(NCORES)))
    ctx = np.concatenate([res.results[c]["ctx_out"] for c in range(NCORES)], axis=0)
    attw = np.concatenate([res.results[c]["attw_out"] for c in range(NCORES)], axis=0)
    return ctx, attw


# revision 8
# speedup vs baseline: 15.0506x; 15.0506x over previous
"""Bass/Trainium2 kernel for the attention module (nn_Attention_18648747999422).

Computes, for B=32, S=2048, ENC=DEC=1024:
    h_proj  = hidden @ w_h.T                     [b, dec]
    e_proj  = encoder_outputs @ w_e.T            [b, s, dec]
    energy  = tanh(h_proj + e_proj + attn_b)
    logits  = energy @ v_w                       [b, s]
    attw    = softmax(where(mask==0, -1e10, logits), axis=s)
    context = attw @ encoder_outputs             [b, enc]
Returns (context, attw).

Data-parallel over 8 NeuronCores: batch dim sharded 4 per core, params
replicated. The big matmul (e_proj) contracts over enc, so encoder_outputs is
cast fp32->fp16 during the DMA load and transposed on-chip with the DMA xbar
(2-byte dtype requirement is why the matmul path runs in fp16; PSUM
accumulation stays fp32, softmax stays fp32).
"""

from contextlib import ExitStack

import numpy as np

B, S, ENC, DEC = 32, 2048, 1024, 1024
NCORES = 8
B_LOC = B // NCORES  # batches per core
P = 128
ET = ENC // P   # 8 enc k-tiles
DT = DEC // P   # 8 dec m-tiles
ST = S // P     # 16 s-tiles
SC = S // 512   # 4 s-chunks of 512
NEG = -1e10

MM_DT_NAME = "float16"   # matmul operand dtype

_CACHE = {}


def _build(repeat=1):
    from concourse import bacc
    import concourse.tile as tile
    import concourse.mybir as mybir

    f32 = mybir.dt.float32
    f16 = getattr(mybir.dt, MM_DT_NAME)
    TANH = mybir.ActivationFunctionType.Tanh
    EXP = mybir.ActivationFunctionType.Exp

    nc = bacc.Bacc("TRN2", target_bir_lowering=False, debug=False)

    enc_in = nc.dram_tensor("enc_in", [B_LOC, S, ENC], f32, kind="ExternalInput")
    hidT_in = nc.dram_tensor("hidT_in", [P, DT, B_LOC], f16, kind="ExternalInput")
    maskb_in = nc.dram_tensor("maskb_in", [B_LOC, S], f32, kind="ExternalInput")
    w_eT_in = nc.dram_tensor("w_eT_in", [P, ET, DEC], f16, kind="ExternalInput")
    w_hT_in = nc.dram_tensor("w_hT_in", [P, DT, DEC], f16, kind="ExternalInput")
    v_wT_in = nc.dram_tensor("v_wT_in", [P, DT], f16, kind="ExternalInput")
    a_bT_in = nc.dram_tensor("a_bT_in", [P, DT], f32, kind="ExternalInput")
    ctx_out = nc.dram_tensor("ctx_out", [B_LOC, ENC], f32, kind="ExternalOutput")
    attw_out = nc.dram_tensor("attw_out", [B_LOC, S], f32, kind="ExternalOutput")

    with tile.TileContext(nc) as tc, ExitStack() as es:
        singles = es.enter_context(tc.tile_pool(name="singles", bufs=1))

        # ---- constants that live for the whole kernel ----
        w_e_sb = singles.tile([P, ET, DEC], f16)
        v_w_sb = singles.tile([P, DT], f16)
        a_bT_sb = singles.tile([P, DT], f32)
        hb_sb = singles.tile([P, DT, B_LOC], f32)

        nc.sync.dma_start(w_e_sb[:], w_eT_in[:])
        nc.sync.dma_start(v_w_sb[:], v_wT_in[:])
        nc.sync.dma_start(a_bT_sb[:], a_bT_in[:])

        # ---- h_proj (scoped pools, freed before the main phase) ----
        # hb[k, dt, b] = sum_d w_h[k,d] hidden[b,d] + attn_b[k]
        # repeat mode keeps the h pools open for the whole kernel (stack
        # order); repeat==1 frees them before the main phase
        hstack = es if repeat != 1 else ExitStack()
        htmp = hstack.enter_context(tc.tile_pool(name="htmp", bufs=1))
        h_pool = hstack.enter_context(
            tc.tile_pool(name="psum_h", bufs=1, space="PSUM"))
        w_h_sb = htmp.tile([P, DT, DEC], f16)
        hidT_sb = htmp.tile([P, DT, B_LOC], f16)
        nc.sync.dma_start(w_h_sb[:], w_hT_in[:])
        nc.sync.dma_start(hidT_sb[:], hidT_in[:])
        psum_h = h_pool.tile([P, DT, B_LOC], f32)
        for kt in range(DT):
            for dh in range(DT):
                nc.tensor.matmul(
                    psum_h[:, kt, :],
                    w_h_sb[:, dh, kt * P:(kt + 1) * P],
                    hidT_sb[:, dh, :],
                    start=(dh == 0),
                    stop=(dh == DT - 1),
                )
        nc.vector.tensor_tensor(
            hb_sb[:],
            psum_h[:],
            a_bT_sb[:, :, None].to_broadcast((P, DT, B_LOC)),
            mybir.AluOpType.add,
        )
        if repeat == 1:
            # free the h-phase pools so the main phase can use the space
            hstack.close()

        # ---- main-phase pools ----
        nat_pool = es.enter_context(tc.tile_pool(name="nat", bufs=2))
        encT_pool = es.enter_context(tc.tile_pool(name="encT", bufs=3))
        energy_pool = es.enter_context(tc.tile_pool(name="energy", bufs=3))
        rows2 = es.enter_context(tc.tile_pool(name="rows2", bufs=2))
        rows1 = es.enter_context(tc.tile_pool(name="rows1", bufs=1))
        pe_pool = es.enter_context(tc.tile_pool(name="psum_e", bufs=3, space="PSUM"))
        logit_pool = es.enter_context(
            tc.tile_pool(name="psum_logit", bufs=2, space="PSUM"))
        ctx_pool = es.enter_context(
            tc.tile_pool(name="psum_ctx", bufs=2, space="PSUM"))

        loop_cm = tc.For_i(0, repeat, 1) if repeat != 1 else None
        if loop_cm is not None:
            loop_cm.__enter__()

        # pending logit matmul (lags one (sc,dt) step so the tanh that feeds
        # it overlaps the next group's e_proj matmuls instead of stalling PE)
        pending = []
        # per-batch deferred softmax+context state
        deferred = []

        def emit_logit(st_):
            sc_, dt_, en_, bstate = st_
            nc.tensor.matmul(
                bstate["psum_logit"][sc_],
                v_w_sb[:, dt_:dt_ + 1],
                en_[:],
                start=(dt_ == 0),
                stop=(dt_ == DT - 1),
            )
            if dt_ == DT - 1:
                # this s-chunk's logits are complete -> copy out of PSUM
                nc.vector.tensor_copy(
                    out=bstate["logits_row"][:, sc_ * 512:(sc_ + 1) * 512],
                    in_=bstate["psum_logit"][sc_],
                )
                bstate["psum_logit"][sc_] = None

        def emit_softmax_ctx(bstate):
            b = bstate["b"]
            logits_row = bstate["logits_row"]
            nat_b16 = bstate["nat_b16"]
            # mask: logits += maskbias (0 or -1e10)
            nc.vector.tensor_tensor(
                logits_row[:], logits_row[:], bstate["maskb_row"][:],
                mybir.AluOpType.add,
            )
            neg_max = rows1.tile([1, 1], f32, name="neg_max", tag="neg_max")
            nc.vector.tensor_reduce(
                neg_max[:], logits_row[:],
                axis=mybir.AxisListType.X, op=mybir.AluOpType.max, negate=True,
            )
            attw_row = rows1.tile([1, S], f32, name="attw_row", tag="attw_row")
            sumexp = rows1.tile([1, 1], f32, name="sumexp", tag="sumexp")
            nc.scalar.activation(
                out=attw_row[:], in_=logits_row[:], func=EXP,
                bias=neg_max[:], scale=1.0, accum_out=sumexp[:],
            )
            rsum = rows1.tile([1, 1], f32, name="rsum", tag="rsum")
            nc.vector.reciprocal(rsum[:], sumexp[:])
            # fp16 copy for the context matmul (scaled), fp32 copy for output
            attw16_row = rows1.tile([1, S], f16, name="attw16_row", tag="attw16_row")
            nc.vector.tensor_scalar_mul(attw16_row[:], attw_row[:], rsum[:])
            nc.vector.tensor_scalar_mul(attw_row[:], attw_row[:], rsum[:])
            nc.scalar.dma_start(attw_out[b:b + 1, :], attw_row[:])
            # attw fp16 [1,2048] -> [16,128] -> xbar transpose -> [128,16].
            # Both on nc.sync: the XBAR is shared hardware, and concurrent
            # use from two HWDGE queues (these vs the enc transposes)
            # corrupts data on HW -- one queue serializes them.
            attw16 = rows1.tile([ST, P], f16, name="attw16", tag="attw16")
            nc.sync.dma_start(
                attw16[:],
                attw16_row[0:1, :].rearrange("p (c j) -> p c j", c=ST),
            )
            attwT = rows1.tile([P, ST], f16, name="attwT", tag="attwT")
            nc.sync.dma_start(attwT[:], attw16[:], transpose=True)
            # context: ctx[n] = sum_s attw[s] * enc[b, s, n]
            ctx_row = rows1.tile([1, ENC], f32, name="ctx_row", tag="ctx_row")
            for nh in range(2):
                psum_ctx = ctx_pool.tile([1, 512], f32, name="psum_ctx",
                                         tag="psum_ctx")
                for st in range(ST):
                    nc.tensor.matmul(
                        psum_ctx[:],
                        attwT[:, st:st + 1],
                        nat_b16[:, st, nh * 512:(nh + 1) * 512],
                        start=(st == 0),
                        stop=(st == ST - 1),
                    )
                nc.vector.tensor_copy(
                    out=ctx_row[:, nh * 512:(nh + 1) * 512], in_=psum_ctx[:]
                )
            nc.scalar.dma_start(ctx_out[b:b + 1, :], ctx_row[:])

        for b in range(B_LOC):
            # ---- loads: cast fp32->fp16 during SWDGE DMA ----
            nat_b16 = nat_pool.tile([P, ST, ENC], f16, name="nat_b16", tag="nat")
            for g in range(2):
                nc.gpsimd.dma_start(
                    out=nat_b16[:, 8 * g:8 * (g + 1), :],
                    in_=enc_in[b, 1024 * g:1024 * (g + 1), :].rearrange(
                        "(st p) e -> p st e", p=P
                    ),
                )
            maskb_row = rows2.tile([1, S], f32, name="maskb_row", tag="maskb_row")
            nc.sync.dma_start(maskb_row[:], maskb_in[b:b + 1, :])

            bstate = {
                "b": b,
                "nat_b16": nat_b16,
                "maskb_row": maskb_row,
                "logits_row": rows2.tile([1, S], f32, name="logits_row",
                                         tag="logits_row"),
                "psum_logit": {},
            }

            # ---- per-s-chunk: xbar transpose then matmuls ----
            for sc in range(SC):
                # one batched xbar transpose per 128-row s-tile:
                #   in  nat[:, st, :]        [128(s), 1024(e)]
                #   out encT_c[:, stl, :, :] [128(e), 8(et), 128(s)] (contiguous)
                encT_c = encT_pool.tile([P, 4, ET, P], f16, name="encT_c",
                                        tag="encT_c")
                for stl in range(4):
                    nc.sync.dma_start(
                        out=encT_c[:, stl, :, :],
                        in_=nat_b16[:, sc * 4 + stl, :],
                        transpose=True,
                    )
                for dt in range(DT):
                    psum_e = pe_pool.tile([P, 512], f32, name="psum_e",
                                          tag="psum_e")
                    for et in range(ET):
                        nc.tensor.matmul(
                            psum_e[:],
                            w_e_sb[:, et, dt * P:(dt + 1) * P],
                            encT_c[:, :, et, :],
                            start=(et == 0),
                            stop=(et == ET - 1),
                        )
                    energy = energy_pool.tile([P, 512], f16, name="energy",
                                              tag="energy")
                    nc.scalar.activation(
                        out=energy[:], in_=psum_e[:], func=TANH,
                        bias=hb_sb[:, dt, b:b + 1], scale=1.0,
                    )
                    if dt == 0:
                        bstate["psum_logit"][sc] = logit_pool.tile(
                            [1, 512], f32, name="psum_logit", tag="psum_logit"
                        )
                    if pending:
                        emit_logit(pending.pop())
                    pending.append((sc, dt, energy, bstate))
                if sc == 0 and deferred:
                    # hide the softmax+ctx latency of the previous batch
                    # behind this batch's matmuls
                    emit_softmax_ctx(deferred.pop())
            deferred.append(bstate)

        while pending:
            emit_logit(pending.pop())
        while deferred:
            emit_softmax_ctx(deferred.pop())

        if loop_cm is not None:
            loop_cm.__exit__(None, None, None)

    nc.compile()
    return nc


def get_nc():
    if "nc" not in _CACHE:
        _CACHE["nc"] = _build()
    return _CACHE["nc"]


def host_prep(hidden, encoder_outputs, mask, attn_w, attn_b, v_w):
    """Shard + lay out inputs for the 8 cores. Returns list of in_maps."""
    np16 = np.float16 if MM_DT_NAME == "float16" else None
    if np16 is None:
        import ml_dtypes
        np16 = ml_dtypes.bfloat16

    def tile_kp(a):  # [K, M] -> [128, K//128, M]
        k, m = a.shape
        return np.ascontiguousarray(
            a.reshape(k // P, P, m).transpose(1, 0, 2)
        )

    w_h = attn_w[:, :DEC]          # [dec(k), dec(d)]
    w_e = attn_w[:, DEC:]          # [dec(k), enc(e)]
    w_hT = tile_kp(np.ascontiguousarray(w_h.T)).astype(np16)   # [128, 8, 1024]
    w_eT = tile_kp(np.ascontiguousarray(w_e.T)).astype(np16)   # [128, 8, 1024]
    v_wT = np.ascontiguousarray(v_w.reshape(DT, P).T).astype(np16)      # [128, 8]
    a_bT = np.ascontiguousarray(attn_b.reshape(DT, P).T).astype(np.float32)

    in_maps = []
    for c in range(NCORES):
        sl = slice(c * B_LOC, (c + 1) * B_LOC)
        hid = hidden[sl]                                       # [4, 1024]
        hidT = tile_kp(np.ascontiguousarray(hid.T)).astype(np16)  # [128, 8, 4]
        maskb = np.where(mask[sl] == 0, np.float32(NEG), np.float32(0.0)).astype(
            np.float32
        )
        in_maps.append({
            "enc_in": np.ascontiguousarray(encoder_outputs[sl]).astype(
                np.float32, copy=False
            ),
            "hidT_in": hidT,
            "maskb_in": maskb,
            "w_eT_in": w_eT,
            "w_hT_in": w_hT,
            "v_wT_in": v_wT,
            "a_bT_in": a_bT,
        })
    return in_maps


def kernel(hidden, encoder_outputs, mask, attn_w, attn_b, v_w):
    from concourse.bass_utils import run_bass_kernel_spmd

    hidden = np.asarray(hidden)
    encoder_outputs = np.asarray(encoder_outputs)
    mask = np.asarray(mask)
    attn_w = np.asarray(attn_w)
    attn_b = np.asarray(attn_b)
    v_w = np.asarray(v_w)

    nc = get_nc()
    in_maps = host_prep(hidden, encoder_outputs, mask, attn_w, attn_b, v_w)
    res = run_bass_kernel_spmd(nc, in_maps, core_ids=list(range(NCORES)))
    ctx = np.concatenate([res.results[c]["ctx_out"] for c in range(NCORES)], axis=0)
    attw = np.concatenate([res.results[c]["attw_out"] for c in range(NCORES)], axis=0)
    return ctx, attw
